# revision 3
# baseline (speedup 1.0000x reference)
"""VQ codebook (CodebookEMA forward) Trainium2 kernel.

Full inputs -> shard batch axis over 8 NeuronCores (2 images/core) ->
Bass/Tile kernel per core -> gather/assemble full outputs.

Per-core pipeline (per batch image, [256, 4096] d-major token matrix):
  1. PE: scores x.w accumulated in PSUM [128 tok, 1024 codes]
     (codebook transposed on-chip once via PE transposes).
  2. DVE custom scan op: single-pass argmax_k of (x.w - |w|^2/2) straight
     out of PSUM (bias row broadcast via gpsimd partition_all_reduce).
  3. GPSIMD ap_gather: zqT[c, tok] = WT[c, idx[tok]] from SBUF-resident
     codebook -> z output is written directly in NCHW layout.
  4. DVE custom reduce: commitment loss partials sum((zq - x)^2).
Host: tiny assembly (loss scalar, perplexity from bincount of idx).
"""
import sys

sys.path.insert(0, "/opt/trn_rl_repo")

import numpy as np
from operator import add

import concourse.bass as bass
import concourse.mybir as mybir
import concourse.tile as tile
import concourse.bass_isa as bass_isa
from concourse import bacc
from concourse.bass_utils import run_bass_kernel_spmd
from concourse.masks import make_identity

# problem constants (hardcoded per contract)
B, C, HH, WW = 16, 256, 64, 64
K = 1024
NCORES = 8
BPC = B // NCORES          # batches per core
TOK = HH * WW              # tokens per batch image
BETA = 0.25
F32 = mybir.dt.float32
F32R = mybir.dt.float32r
U16 = mybir.dt.uint16
I32 = mybir.dt.int32
I16 = mybir.dt.int16

# ---------------------------------------------------------------- custom ops
_OPS = {}


def _register_ops():
    if _OPS:
        return _OPS
    import concourse.dve_ops as dve_ops
    from concourse.dve_ops import DveOp
    from concourse.dve_spec import (
        Spec, Src0, Src1, C0, Idx, AluOp, MaxNeg, eq, select, maxx, sq, Scan,
        lower, _has_src1,
    )
    from concourse.dve_uop import DveOpSpec

    def make_op(name, spec, subdim=False):
        existing = {o.name: o for o in dve_ops.OPS}
        if name in existing:
            return existing[name]
        opcode = dve_ops._CUSTOM_DVE_ROW_BASE + len(dve_ops.OPS)
        shas = {}
        for ver in ("v3", "v4"):
            shas[ver] = DveOpSpec(
                name=name, opcode=opcode, uops=lower(spec, ver=ver),
                rd1_en=_has_src1(spec),
            ).sha(ver)
        op = DveOp(name, spec, subdim=subdim, uops_sha=shas)
        dve_ops.OPS.append(op)
        dve_ops.CUSTOM_DVE_SPECS[name] = spec
        dve_ops._SUB_OPCODE_FOR_NAME[name] = opcode
        return op

    def _ref_argmax_scan(in0, in1, s0, s1, imm2):
        b = (in0.astype(np.float32) - in1.astype(np.float32)).astype(np.float32)
        r = np.maximum.accumulate(b, axis=-1)
        n = b.shape[-1]
        idxs = np.arange(n, dtype=np.float32)
        marked = np.where(b == r, idxs, -np.finfo(np.float32).max)
        return marked, marked.reshape(marked.shape[0], -1).max(axis=-1, keepdims=True)

    def _ref_subsq_reduce(in0, in1, s0, s1, imm2):
        b = (in0.astype(np.float32) - in1.astype(np.float32)) ** 2
        return b.astype(np.float32), (
            s0 + b.reshape(b.shape[0], -1).sum(axis=-1, keepdims=True)
        )

    _b = Src0 - Src1
    _r = Scan(AluOp.MAX, _b)
    _OPS["argmax"] = make_op(
        "ARGMAX_SCAN_VQ",
        Spec(body=select(eq(_b, _r), Idx, MaxNeg), accum=maxx,
             reference=_ref_argmax_scan),
    )
    _OPS["subsq"] = make_op(
        "SUBSQ_REDUCE_VQ",
        Spec(body=sq(Src0 - Src1), accum=add, accum_init=C0,
             reference=_ref_subsq_reduce),
    )
    return _OPS


# ---------------------------------------------------------------- kernel build
_NC_CACHE = {}


def _build():
    if "nc" in _NC_CACHE:
        return _NC_CACHE["nc"]
    ops = _register_ops()
    nc = bacc.Bacc("TRN2", target_bir_lowering=False, debug=False,
                   num_devices=NCORES)

    x_in = nc.declare_dram_parameter("x", [BPC, C, TOK], F32, isOutput=False)
    cb_in = nc.declare_dram_parameter("cb", [K, C], F32, isOutput=False)
    z_out = nc.declare_dram_parameter("z", [BPC, C, TOK], F32, isOutput=True)
    idx_out = nc.declare_dram_parameter("idx", [BPC, TOK], I32, isOutput=True)
    stats_out = nc.declare_dram_parameter("stats", [128, 1], F32, isOutput=True)
    idx16_dram = nc.dram_tensor("idx16", [BPC, TOK], U16)

    NTT = TOK // 128  # token tiles per batch (32)

    with tile.TileContext(nc) as tc:
        with tc.tile_pool(name="sbuf", bufs=1) as pool, \
             tc.tile_pool(name="psum", bufs=1, space="PSUM") as psum:
            # ---------------- setup: identity, codebook transpose, bias row
            ident = pool.tile([128, 128], F32, name="ident")
            make_identity(nc, ident[:])

            wk = pool.tile([128, 8 * C], F32, name="wk")
            nc.sync.dma_start(
                out=wk[:].rearrange("p (kt d) -> p kt d", kt=8),
                in_=cb_in[:].rearrange("(kt p) d -> p kt d", p=128),
            )
            wk3 = wk[:].rearrange("p (kt d) -> p kt d", kt=8)

            wt = [pool.tile([128, K], F32, name=f"wt{dt}") for dt in range(2)]
            for kt in range(8):
                for dt in range(2):
                    pst = psum.tile([128, 128], F32, tag="pst", name="pst",
                                    bufs=1)
                    nc.tensor.transpose(
                        pst[:], wk3[:, kt, dt * 128:(dt + 1) * 128], ident[:])
                    nc.scalar.activation(
                        wt[dt][:, kt * 128:(kt + 1) * 128], pst[:],
                        mybir.ActivationFunctionType.Copy)

            sq_a = pool.tile([128, K], F32, name="sq_a")
            sq_b = pool.tile([128, K], F32, name="sq_b")
            bias = pool.tile([128, K], F32, name="bias")
            nc.vector.tensor_mul(sq_a[:], wt[0][:], wt[0][:])
            nc.vector.tensor_mul(sq_b[:], wt[1][:], wt[1][:])
            nc.vector.tensor_add(sq_a[:], sq_a[:], sq_b[:])
            nc.gpsimd.partition_all_reduce(sq_b[:], sq_a[:], channels=128,
                                           reduce_op=bass_isa.ReduceOp.add)
            nc.vector.tensor_scalar_mul(bias[:], sq_b[:], 0.5)

            stats_ssq = pool.tile([128, 2 * BPC], F32, name="stats_ssq")

            # ---------------- main loop over the core's batch images
            for b in range(BPC):
                xb = [pool.tile([128, TOK], F32, tag=f"xb{dt}",
                                name=f"xb{dt}_{b}", bufs=2) for dt in range(2)]
                for dt in range(2):
                    nc.sync.dma_start(out=xb[dt][:],
                                      in_=x_in[b, dt * 128:(dt + 1) * 128, :])

                stats_idx = pool.tile([128, NTT], F32, tag="sidx",
                                      name=f"sidx_{b}", bufs=2)
                for tt in range(NTT):
                    ps = psum.tile([128, K], F32, tag="ps", name=f"ps_{b}_{tt}",
                                   bufs=3)
                    for kc in range(2):
                        for dt in range(2):
                            nc.tensor.matmul(
                                ps[:, kc * 512:(kc + 1) * 512],
                                xb[dt][:, tt * 128:(tt + 1) * 128],
                                wt[dt][:, kc * 512:(kc + 1) * 512],
                                start=(dt == 0), stop=(dt == 1),
                            )
                    scr = pool.tile([128, K], F32, tag="scr",
                                    name=f"scr_{b}_{tt}", bufs=3)
                    nc.vector._custom_dve(
                        ops["argmax"], out=scr[:], in0=ps[:], in1=bias[:],
                        accum_out=stats_idx[:, tt:tt + 1])

                # idx postprocessing: [128 tok, 32 tile] -> token-major [4096]
                psi = psum.tile([NTT, 128], F32, tag="psi", name=f"psi_{b}",
                                bufs=1)
                nc.tensor.transpose(psi[:], stats_idx[:], ident[:])
                idxT16 = pool.tile([NTT, 128], U16, tag="idxT16",
                                   name=f"idxT16_{b}", bufs=2)
                idxT32 = pool.tile([NTT, 128], I32, tag="idxT32",
                                   name=f"idxT32_{b}", bufs=2)
                nc.vector.tensor_copy(idxT16[:], psi[:])
                nc.vector.tensor_copy(idxT32[:], psi[:])
                nc.sync.dma_start(
                    out=idx_out[b].rearrange("(t p) -> t p", p=128),
                    in_=idxT32[:])
                nc.sync.dma_start(
                    out=idx16_dram[b].rearrange("(t p) -> t p", p=128),
                    in_=idxT16[:])

                # wrapped-idx layout for ap_gather: [16q x TOK/16], replicated
                # into each 16-partition group
                idxw = pool.tile([128, TOK // 16], U16, tag="idxw",
                                 name=f"idxw_{b}", bufs=2)
                wrapped_src = idx16_dram[b].rearrange("(j q) -> q j", q=16)
                for g in range(8):
                    nc.sync.dma_start(out=idxw[16 * g:16 * (g + 1), :],
                                      in_=wrapped_src)

                # gather zqT = WT[:, idx] per channel tile; loss partials; out
                idxw_i16 = idxw[:].bitcast(I16)
                for ct in range(2):
                    zq = pool.tile([128, TOK], F32, tag=f"zq{ct}",
                                   name=f"zq{ct}_{b}", bufs=2)
                    nc.gpsimd.ap_gather(
                        zq[:].rearrange("p (t o) -> p t o", o=1),
                        wt[ct][:].rearrange("p (k o) -> p k o", o=1),
                        idxw_i16,
                        channels=128, num_elems=K, d=1, num_idxs=TOK)
                    nc.sync.dma_start(
                        out=z_out[b, ct * 128:(ct + 1) * 128, :], in_=zq[:])
                    nc.vector._custom_dve(
                        ops["subsq"], out=zq[:], in0=zq[:], in1=xb[ct][:],
                        s0=0.0, accum_out=stats_ssq[:, 2 * b + ct:2 * b + ct + 1])

            stats_red = pool.tile([128, 1], F32, name="stats_red")
            nc.vector.reduce_sum(stats_red[:], stats_ssq[:],
                                 axis=mybir.AxisListType.X)
            nc.sync.dma_start(out=stats_out[:], in_=stats_red[:])

    nc.compile()
    _NC_CACHE["nc"] = nc
    return nc


# ---------------------------------------------------------------- entry point
def kernel(inputs: np.ndarray, codebook: np.ndarray):
    inputs = np.ascontiguousarray(np.asarray(inputs, dtype=np.float32))
    codebook = np.ascontiguousarray(np.asarray(codebook, dtype=np.float32))
    nc = _build()

    x_flat = inputs.reshape(B, C, TOK)
    in_maps = [
        {"x": np.ascontiguousarray(x_flat[c * BPC:(c + 1) * BPC]),
         "cb": codebook}
        for c in range(NCORES)
    ]
    _r = run_bass_kernel_spmd(nc, in_maps, list(range(NCORES)))
    globals()["LAST_RESULTS"] = _r
    res = _r.results

    z = np.concatenate([r["z"] for r in res], axis=0).reshape(B, C, HH, WW)
    idx = np.concatenate([r["idx"].reshape(-1) for r in res]).astype(np.int32)
    ssq = float(sum(r["stats"].astype(np.float64).sum() for r in res))

    n_tokens = B * TOK
    loss = np.float32(BETA * ssq / (n_tokens * C))
    counts = np.bincount(idx, minlength=K).astype(np.float64)
    avg = counts / n_tokens
    perplexity = np.float32(np.exp(-np.sum(avg * np.log(avg + 1e-10))))
    return z, loss, perplexity, idx[:, None].astype(np.int32)


# revision 6
# speedup vs baseline: 1.1893x; 1.1893x over previous
"""VQ codebook (CodebookEMA forward) Trainium2 kernel.

Full inputs -> shard batch axis over 8 NeuronCores (2 images/core) ->
Bass/Tile kernel per core -> gather/assemble full outputs.

Per-core pipeline (per batch image, [256, 4096] d-major token matrix):
  1. PE: scores x.w accumulated in PSUM [128 tok, 1024 codes]
     (codebook transposed on-chip once via PE transposes). Token tiles
     are built STRIDED (token n = 512r + 16t + q lands at partition
     16r + q of tile t) so the per-tile argmax column [128, 1] is, per
     16-partition group, already the "wrapped-16" index layout that the
     GPSIMD gather expects.
  2. DVE custom scan op: single-pass argmax_k of (x.w - |w|^2/2) read
     straight out of PSUM (bias row broadcast by partition_all_reduce).
  3. GPSIMD ap_gather: zqT[c, tok] = WT[c, idx[tok]] from SBUF-resident
     transposed codebook -> z output written directly in NCHW layout.
  4. DVE custom reduce: commitment loss partials sum((zq - x)^2).
Host: tiny assembly (idx de-permute, loss scalar, perplexity bincount).
"""
import sys

sys.path.insert(0, "/opt/trn_rl_repo")

import numpy as np
from operator import add

import concourse.bass as bass
import concourse.mybir as mybir
import concourse.tile as tile
import concourse.bass_isa as bass_isa
from concourse import bacc
from concourse.bass_utils import run_bass_kernel_spmd
from concourse.masks import make_identity

# problem constants (hardcoded per contract)
B, C, HH, WW = 16, 256, 64, 64
K = 1024
NCORES = 8
BPC = B // NCORES          # batches per core
TOK = HH * WW              # tokens per batch image
BETA = 0.25
F32 = mybir.dt.float32
F32R = mybir.dt.float32r
U16 = mybir.dt.uint16
I32 = mybir.dt.int32
I16 = mybir.dt.int16

NTT = TOK // 128           # token tiles per batch (32)
NR = 8                     # partition sub-groups (r) per tile
NQ = 16                    # wrapped group width (q)

# ---------------------------------------------------------------- custom ops
_OPS = {}


def _register_ops():
    if _OPS:
        return _OPS
    import concourse.dve_ops as dve_ops
    from concourse.dve_ops import DveOp
    from concourse.dve_spec import (
        Spec, Src0, Src1, C0, Idx, AluOp, MaxNeg, eq, select, maxx, sq, Scan,
        lower, _has_src1,
    )
    from concourse.dve_uop import DveOpSpec

    def make_op(name, spec, subdim=False):
        existing = {o.name: o for o in dve_ops.OPS}
        if name in existing:
            return existing[name]
        opcode = dve_ops._CUSTOM_DVE_ROW_BASE + len(dve_ops.OPS)
        shas = {}
        for ver in ("v3", "v4"):
            shas[ver] = DveOpSpec(
                name=name, opcode=opcode, uops=lower(spec, ver=ver),
                rd1_en=_has_src1(spec),
            ).sha(ver)
        op = DveOp(name, spec, subdim=subdim, uops_sha=shas)
        dve_ops.OPS.append(op)
        dve_ops.CUSTOM_DVE_SPECS[name] = spec
        dve_ops._SUB_OPCODE_FOR_NAME[name] = opcode
        return op

    def _ref_argmax_scan(in0, in1, s0, s1, imm2):
        b = (in0.astype(np.float32) - in1.astype(np.float32)).astype(np.float32)
        r = np.maximum.accumulate(b, axis=-1)
        n = b.shape[-1]
        idxs = np.arange(n, dtype=np.float32)
        marked = np.where(b == r, idxs, -np.finfo(np.float32).max)
        return marked, marked.reshape(marked.shape[0], -1).max(axis=-1, keepdims=True)

    def _ref_subsq_reduce(in0, in1, s0, s1, imm2):
        b = (in0.astype(np.float32) - in1.astype(np.float32)) ** 2
        return b.astype(np.float32), (
            s0 + b.reshape(b.shape[0], -1).sum(axis=-1, keepdims=True)
        )

    _b = Src0 - Src1
    _r = Scan(AluOp.MAX, _b)
    _OPS["argmax"] = make_op(
        "ARGMAX_SCAN_VQ",
        Spec(body=select(eq(_b, _r), Idx, MaxNeg), accum=maxx,
             reference=_ref_argmax_scan),
    )
    _OPS["subsq"] = make_op(
        "SUBSQ_REDUCE_VQ",
        Spec(body=sq(Src0 - Src1), accum=add, accum_init=C0,
             reference=_ref_subsq_reduce),
    )
    return _OPS


# ---------------------------------------------------------------- kernel build
_NC_CACHE = {}


def _build():
    if "nc" in _NC_CACHE:
        return _NC_CACHE["nc"]
    ops = _register_ops()
    nc = bacc.Bacc("TRN2", target_bir_lowering=False, debug=False,
                   num_devices=NCORES)

    x_in = nc.declare_dram_parameter("x", [BPC, C, TOK], F32, isOutput=False)
    cb_in = nc.declare_dram_parameter("cb", [K, C], F32, isOutput=False)
    z_out = nc.declare_dram_parameter("z", [BPC, C, TOK], F32, isOutput=True)
    idx_out = nc.declare_dram_parameter("idx", [BPC, TOK], I32, isOutput=True)
    stats_out = nc.declare_dram_parameter("stats", [128, 1], F32, isOutput=True)

    with tile.TileContext(nc) as tc:
        with tc.tile_pool(name="sbuf", bufs=1) as pool, \
             tc.tile_pool(name="psum", bufs=1, space="PSUM") as psum:
            # ---------------- setup: identity, codebook transpose, bias row
            ident = pool.tile([128, 128], F32, name="ident")
            make_identity(nc, ident[:])

            wk = pool.tile([128, 8 * C], F32, name="wk")
            nc.sync.dma_start(
                out=wk[:].rearrange("p (kt d) -> p kt d", kt=8),
                in_=cb_in[:].rearrange("(kt p) d -> p kt d", p=128),
            )
            wk3 = wk[:].rearrange("p (kt d) -> p kt d", kt=8)

            wt = [pool.tile([128, K], F32, name=f"wt{dt}") for dt in range(2)]
            for kt in range(8):
                for dt in range(2):
                    pst = psum.tile([128, 128], F32, tag="pst", name="pst",
                                    bufs=1)
                    nc.tensor.transpose(
                        pst[:], wk3[:, kt, dt * 128:(dt + 1) * 128], ident[:])
                    nc.scalar.activation(
                        wt[dt][:, kt * 128:(kt + 1) * 128], pst[:],
                        mybir.ActivationFunctionType.Copy)

            sq_a = pool.tile([128, K], F32, name="sq_a")
            sq_b = pool.tile([128, K], F32, name="sq_b")
            bias = pool.tile([128, K], F32, name="bias")
            nc.vector.tensor_mul(sq_a[:], wt[0][:], wt[0][:])
            nc.vector.tensor_mul(sq_b[:], wt[1][:], wt[1][:])
            nc.vector.tensor_add(sq_a[:], sq_a[:], sq_b[:])
            nc.gpsimd.partition_all_reduce(sq_b[:], sq_a[:], channels=128,
                                           reduce_op=bass_isa.ReduceOp.add)
            nc.vector.tensor_scalar_mul(bias[:], sq_b[:], 0.5)

            stats_ssq = pool.tile([128, 2 * BPC], F32, name="stats_ssq")

            # ---------------- phase 1 per batch: score + argmax + idx plumbing
            xb_all, idxw_all = [], []
            for b in range(BPC):
                xb = [pool.tile([128, TOK], F32, tag=f"xb{dt}",
                                name=f"xb{dt}_{b}", bufs=2) for dt in range(2)]
                xb_all.append(xb)
                for dt in range(2):
                    nc.sync.dma_start(out=xb[dt][:],
                                      in_=x_in[b, dt * 128:(dt + 1) * 128, :])

                stats_idx = pool.tile([128, NTT], F32, tag="sidx",
                                      name=f"sidx_{b}", bufs=2)
                for tt in range(NTT):
                    ps = psum.tile([128, K], F32, tag="ps", name=f"ps_{b}_{tt}",
                                   bufs=3)
                    for kc in range(2):
                        for dt in range(2):
                            nc.tensor.matmul(
                                ps[:, kc * 512:(kc + 1) * 512],
                                xb[dt][:, tt * 128:(tt + 1) * 128],
                                wt[dt][:, kc * 512:(kc + 1) * 512],
                                start=(dt == 0), stop=(dt == 1),
                            )
                    scr = pool.tile([128, K], F32, tag="scr",
                                    name=f"scr_{b}_{tt}", bufs=3)
                    nc.vector._custom_dve(
                        ops["argmax"], out=scr[:], in0=ps[:], in1=bias[:],
                        accum_out=stats_idx[:, tt:tt + 1])

                # idx -> token-major: psi[t, p] = idx(token 128t + p)
                psi = psum.tile([NTT, 128], F32, tag="psi", name=f"psi_{b}",
                                bufs=1)
                nc.tensor.transpose(psi[:], stats_idx[:], ident[:])
                idxT32 = pool.tile([NTT, 128], I32, tag="idxT32",
                                   name=f"idxT32_{b}", bufs=2)
                nc.vector.tensor_copy(idxT32[:], psi[:])
                nc.sync.dma_start(
                    out=idx_out[b].rearrange("(t p) -> t p", p=128),
                    in_=idxT32[:])
                psi_sb = pool.tile([NTT, 128], F32, tag="psi_sb",
                                   name=f"psi_sb_{b}", bufs=2)
                nc.scalar.activation(psi_sb[:], psi[:],
                                     mybir.ActivationFunctionType.Copy)

                # wrapped-16 idx for ap_gather: idxw[16g+q, 8t+r] = idx of
                # token 128t + 16r + q; built from 8 tiny transposes of
                # psi_sb[:, 16r:16r+16] -> [16 q, 32 t] written at stride 8.
                idxw = pool.tile([128, TOK // 16], U16, tag="idxw",
                                 name=f"idxw_{b}", bufs=2)
                idxw_all.append(idxw)
                iw3 = idxw[:].rearrange("p (t r) -> p t r", r=NR)
                for r in range(NR):
                    pst2 = psum.tile([16, NTT], F32, tag="pst",
                                     name=f"pst2_{b}_{r}", bufs=1)
                    nc.tensor.transpose(
                        pst2[:], psi_sb[:, 16 * r:16 * (r + 1)],
                        ident[0:NTT, 0:NTT])
                    nc.vector.tensor_copy(iw3[0:16, :, r], pst2[:])
                for g in range(1, 8):
                    nc.sync.dma_start(out=idxw[16 * g:16 * (g + 1), :],
                                      in_=idxw[0:16, :])

            # ---------------- phase 2 per batch: gather, z out, loss partials
            for b in range(BPC):
                idxw_i16 = idxw_all[b][:].bitcast(I16)
                for ct in range(2):
                    zq = pool.tile([128, TOK], F32, tag=f"zq{ct}",
                                   name=f"zq{ct}_{b}", bufs=2)
                    nc.gpsimd.ap_gather(
                        zq[:].rearrange("p (t o) -> p t o", o=1),
                        wt[ct][:].rearrange("p (k o) -> p k o", o=1),
                        idxw_i16,
                        channels=128, num_elems=K, d=1, num_idxs=TOK)
                    nc.sync.dma_start(
                        out=z_out[b, ct * 128:(ct + 1) * 128, :], in_=zq[:])
                    nc.vector._custom_dve(
                        ops["subsq"], out=zq[:], in0=zq[:],
                        in1=xb_all[b][ct][:], s0=0.0,
                        accum_out=stats_ssq[:, 2 * b + ct:2 * b + ct + 1])

            stats_red = pool.tile([128, 1], F32, name="stats_red")
            nc.vector.reduce_sum(stats_red[:], stats_ssq[:],
                                 axis=mybir.AxisListType.X)
            nc.sync.dma_start(out=stats_out[:], in_=stats_red[:])

    nc.compile()
    _NC_CACHE["nc"] = nc
    return nc


# ---------------------------------------------------------------- entry point
def kernel(inputs: np.ndarray, codebook: np.ndarray):
    inputs = np.ascontiguousarray(np.asarray(inputs, dtype=np.float32))
    codebook = np.ascontiguousarray(np.asarray(codebook, dtype=np.float32))
    nc = _build()

    x_flat = inputs.reshape(B, C, TOK)
    in_maps = [
        {"x": np.ascontiguousarray(x_flat[c * BPC:(c + 1) * BPC]),
         "cb": codebook}
        for c in range(NCORES)
    ]
    _r = run_bass_kernel_spmd(nc, in_maps, list(range(NCORES)))
    globals()["LAST_RESULTS"] = _r
    res = _r.results

    z = np.concatenate([r["z"] for r in res], axis=0).reshape(B, C, HH, WW)
    idx = np.concatenate([r["idx"].reshape(-1) for r in res]).astype(np.int32)
    ssq = float(sum(r["stats"].astype(np.float64).sum() for r in res))

    n_tokens = B * TOK
    loss = np.float32(BETA * ssq / (n_tokens * C))
    counts = np.bincount(idx, minlength=K).astype(np.float64)
    avg = counts / n_tokens
    perplexity = np.float32(np.exp(-np.sum(avg * np.log(avg + 1e-10))))
    return z, loss, perplexity, idx[:, None].astype(np.int32)


# revision 8
# speedup vs baseline: 2.3998x; 2.0179x over previous
"""VQ codebook (CodebookEMA forward) Trainium2 kernel.

Full inputs -> shard batch axis over 8 NeuronCores (2 images/core) ->
Bass/Tile kernel per core -> gather/assemble full outputs.

Per-core pipeline (per batch image, [256, 4096] d-major token matrix):
  1. PE: scores x.w accumulated in PSUM [128 tok, 1024 codes]
     (codebook transposed on-chip once via PE transposes). Token tiles
     run [128 tokens x 1024 codes] per PSUM tile.
  2. DVE custom scan op: single-pass argmax_k of (x.w - |w|^2/2) read
     straight out of PSUM (bias row broadcast by partition_all_reduce).
  3. GPSIMD indirect DMA: gather codebook rows by token index from DRAM
     ([tok, d] tiles), then PE-transpose to zqT [c, tok] NCHW layout.
  4. DVE custom reduce: commitment loss partials sum((zq - x)^2).
Host: tiny assembly (idx de-permute, loss scalar, perplexity bincount).
"""
import sys

sys.path.insert(0, "/opt/trn_rl_repo")

import numpy as np
from operator import add

import concourse.bass as bass
import concourse.mybir as mybir
import concourse.tile as tile
import concourse.bass_isa as bass_isa
from concourse import bacc
from concourse.bass_utils import run_bass_kernel_spmd
from concourse.masks import make_identity

# problem constants (hardcoded per contract)
B, C, HH, WW = 16, 256, 64, 64
K = 1024
NCORES = 8
BPC = B // NCORES          # batches per core
TOK = HH * WW              # tokens per batch image
BETA = 0.25
F32 = mybir.dt.float32
F32R = mybir.dt.float32r
U16 = mybir.dt.uint16
I32 = mybir.dt.int32
I16 = mybir.dt.int16

NTT = TOK // 128           # token tiles per batch (32)
NR = 8                     # partition sub-groups (r) per tile
NQ = 16                    # wrapped group width (q)
GCH = 16                   # token tiles per indirect-gather chunk

# ---------------------------------------------------------------- custom ops
_OPS = {}


def _register_ops():
    if _OPS:
        return _OPS
    import concourse.dve_ops as dve_ops
    from concourse.dve_ops import DveOp
    from concourse.dve_spec import (
        Spec, Src0, Src1, C0, Idx, AluOp, MaxNeg, eq, select, maxx, sq, Scan,
        lower, _has_src1,
    )
    from concourse.dve_uop import DveOpSpec

    def make_op(name, spec, subdim=False):
        existing = {o.name: o for o in dve_ops.OPS}
        if name in existing:
            return existing[name]
        opcode = dve_ops._CUSTOM_DVE_ROW_BASE + len(dve_ops.OPS)
        shas = {}
        for ver in ("v3", "v4"):
            shas[ver] = DveOpSpec(
                name=name, opcode=opcode, uops=lower(spec, ver=ver),
                rd1_en=_has_src1(spec),
            ).sha(ver)
        op = DveOp(name, spec, subdim=subdim, uops_sha=shas)
        dve_ops.OPS.append(op)
        dve_ops.CUSTOM_DVE_SPECS[name] = spec
        dve_ops._SUB_OPCODE_FOR_NAME[name] = opcode
        return op

    def _ref_argmax_scan(in0, in1, s0, s1, imm2):
        b = (in0.astype(np.float32) - in1.astype(np.float32)).astype(np.float32)
        r = np.maximum.accumulate(b, axis=-1)
        n = b.shape[-1]
        idxs = np.arange(n, dtype=np.float32)
        marked = np.where(b == r, idxs, -np.finfo(np.float32).max)
        return marked, marked.reshape(marked.shape[0], -1).max(axis=-1, keepdims=True)

    def _ref_subsq_reduce(in0, in1, s0, s1, imm2):
        b = (in0.astype(np.float32) - in1.astype(np.float32)) ** 2
        return b.astype(np.float32), (
            s0 + b.reshape(b.shape[0], -1).sum(axis=-1, keepdims=True)
        )

    _b = Src0 - Src1
    _r = Scan(AluOp.MAX, _b)
    _OPS["argmax"] = make_op(
        "ARGMAX_SCAN_VQ",
        Spec(body=select(eq(_b, _r), Idx, MaxNeg), accum=maxx,
             reference=_ref_argmax_scan),
    )
    _OPS["subsq"] = make_op(
        "SUBSQ_REDUCE_VQ",
        Spec(body=sq(Src0 - Src1), accum=add, accum_init=C0,
             reference=_ref_subsq_reduce),
    )
    return _OPS


# ---------------------------------------------------------------- kernel build
_NC_CACHE = {}


def _build():
    if "nc" in _NC_CACHE:
        return _NC_CACHE["nc"]
    ops = _register_ops()
    nc = bacc.Bacc("TRN2", target_bir_lowering=False, debug=False,
                   num_devices=NCORES)

    x_in = nc.declare_dram_parameter("x", [BPC, C, TOK], F32, isOutput=False)
    cb_in = nc.declare_dram_parameter("cb", [K, C], F32, isOutput=False)
    z_out = nc.declare_dram_parameter("z", [BPC, C, TOK], F32, isOutput=True)
    idx_out = nc.declare_dram_parameter("idx", [BPC, TOK], I32, isOutput=True)
    stats_out = nc.declare_dram_parameter("stats", [128, 1], F32, isOutput=True)

    with tile.TileContext(nc) as tc:
        with tc.tile_pool(name="sbuf", bufs=1) as pool, \
             tc.tile_pool(name="psum", bufs=1, space="PSUM") as psum:
            # ---------------- setup: identity, codebook transpose, bias row
            ident = pool.tile([128, 128], F32, name="ident")
            make_identity(nc, ident[:])

            wk = pool.tile([128, 8 * C], F32, tag="gbuf", name="wk", bufs=4)
            nc.sync.dma_start(
                out=wk[:].rearrange("p (kt d) -> p kt d", kt=8),
                in_=cb_in[:].rearrange("(kt p) d -> p kt d", p=128),
            )
            wk3 = wk[:].rearrange("p (kt d) -> p kt d", kt=8)

            wt = [pool.tile([128, K], F32, name=f"wt{dt}") for dt in range(2)]
            for kt in range(8):
                for dt in range(2):
                    pst = psum.tile([128, 128], F32, tag="pst", name="pst",
                                    bufs=1)
                    nc.tensor.transpose(
                        pst[:], wk3[:, kt, dt * 128:(dt + 1) * 128], ident[:])
                    nc.scalar.activation(
                        wt[dt][:, kt * 128:(kt + 1) * 128], pst[:],
                        mybir.ActivationFunctionType.Copy)

            sq_a = pool.tile([128, K], F32, name="sq_a")
            sq_b = pool.tile([128, K], F32, name="sq_b")
            bias = pool.tile([128, K], F32, name="bias")
            nc.vector.tensor_mul(sq_a[:], wt[0][:], wt[0][:])
            nc.vector.tensor_mul(sq_b[:], wt[1][:], wt[1][:])
            nc.vector.tensor_add(sq_a[:], sq_a[:], sq_b[:])
            nc.gpsimd.partition_all_reduce(sq_b[:], sq_a[:], channels=128,
                                           reduce_op=bass_isa.ReduceOp.add)
            nc.vector.tensor_scalar_mul(bias[:], sq_b[:], 0.5)

            stats_ssq = pool.tile([128, 2 * BPC], F32, name="stats_ssq")

            # ---------------- phase 1 per batch: score + argmax + idx plumbing
            xb_all, gbuf_all = [], {}
            for b in range(BPC):
                xb = [pool.tile([128, TOK], F32, tag=f"xb{dt}",
                                name=f"xb{dt}_{b}", bufs=2) for dt in range(2)]
                xb_all.append(xb)
                for dt in range(2):
                    nc.sync.dma_start(out=xb[dt][:],
                                      in_=x_in[b, dt * 128:(dt + 1) * 128, :])

                stats_idx = pool.tile([128, NTT], F32, tag="sidx",
                                      name=f"sidx_{b}", bufs=2)
                stats_i32 = pool.tile([128, NTT], I32, tag="sidx32",
                                      name=f"sidx32_{b}", bufs=2)
                for tt in range(NTT):
                    ps = psum.tile([128, K], F32, tag="ps", name=f"ps_{b}_{tt}",
                                   bufs=2)
                    for kc in range(2):
                        for dt in range(2):
                            nc.tensor.matmul(
                                ps[:, kc * 512:(kc + 1) * 512],
                                xb[dt][:, tt * 128:(tt + 1) * 128],
                                wt[dt][:, kc * 512:(kc + 1) * 512],
                                start=(dt == 0), stop=(dt == 1),
                            )
                    scr = pool.tile([128, K], F32, tag="scr",
                                    name=f"scr_{b}_{tt}", bufs=3)
                    nc.vector._custom_dve(
                        ops["argmax"], out=scr[:], in0=ps[:], in1=bias[:],
                        accum_out=stats_idx[:, tt:tt + 1])
                    if tt % GCH == GCH - 1:
                        cc = tt // GCH
                        sl = slice(cc * GCH, (cc + 1) * GCH)
                        nc.vector.tensor_copy(stats_i32[:, sl],
                                              stats_idx[:, sl])
                        gb = pool.tile([128, GCH * C], F32, tag="gbuf",
                                       name=f"gb_{b}_{cc}", bufs=4)
                        gbuf_all[(b, cc)] = gb
                        g3 = gb[:].rearrange("p (t d) -> p t d", d=C)
                        for j in range(GCH):
                            nc.gpsimd.indirect_dma_start(
                                out=g3[:, j],
                                out_offset=None,
                                in_=cb_in[:],
                                in_offset=bass.IndirectOffsetOnAxis(
                                    ap=stats_i32[:, cc * GCH + j:
                                                 cc * GCH + j + 1], axis=0),
                            )

                # idx -> token-major: psi[t, p] = idx(token 128t + p)
                psi = psum.tile([NTT, 128], F32, tag="psi", name=f"psi_{b}",
                                bufs=1)
                nc.tensor.transpose(psi[:], stats_idx[:], ident[:])
                idxT32 = pool.tile([NTT, 128], I32, tag="idxT32",
                                   name=f"idxT32_{b}", bufs=2)
                nc.vector.tensor_copy(idxT32[:], psi[:])
                nc.sync.dma_start(
                    out=idx_out[b].rearrange("(t p) -> t p", p=128),
                    in_=idxT32[:])

            # ---------------- phase 2 per batch: transpose, z out, loss
            for b in range(BPC):
                zq = [pool.tile([128, TOK], F32, tag=f"zq{ct}",
                                name=f"zq{ct}_{b}", bufs=1) for ct in range(2)]
                for tt in range(NTT):
                    gb = gbuf_all[(b, tt // GCH)]
                    g3 = gb[:].rearrange("p (t d) -> p t d", d=C)
                    for ct in range(2):
                        pz = psum.tile([128, 128], F32, tag="pz",
                                       name=f"pz_{b}_{tt}_{ct}", bufs=2)
                        nc.tensor.transpose(
                            pz[:], g3[:, tt % GCH, ct * 128:(ct + 1) * 128],
                            ident[:])
                        nc.scalar.activation(
                            zq[ct][:, tt * 128:(tt + 1) * 128], pz[:],
                            mybir.ActivationFunctionType.Copy)
                for ct in range(2):
                    nc.sync.dma_start(
                        out=z_out[b, ct * 128:(ct + 1) * 128, :], in_=zq[ct][:])
                    nc.vector._custom_dve(
                        ops["subsq"], out=zq[ct][:], in0=zq[ct][:],
                        in1=xb_all[b][ct][:], s0=0.0,
                        accum_out=stats_ssq[:, 2 * b + ct:2 * b + ct + 1])

            stats_red = pool.tile([128, 1], F32, name="stats_red")
            nc.vector.reduce_sum(stats_red[:], stats_ssq[:],
                                 axis=mybir.AxisListType.X)
            nc.sync.dma_start(out=stats_out[:], in_=stats_red[:])

    nc.compile()
    _NC_CACHE["nc"] = nc
    return nc


# ---------------------------------------------------------------- entry point
def kernel(inputs: np.ndarray, codebook: np.ndarray):
    inputs = np.ascontiguousarray(np.asarray(inputs, dtype=np.float32))
    codebook = np.ascontiguousarray(np.asarray(codebook, dtype=np.float32))
    nc = _build()

    x_flat = inputs.reshape(B, C, TOK)
    in_maps = [
        {"x": np.ascontiguousarray(x_flat[c * BPC:(c + 1) * BPC]),
         "cb": codebook}
        for c in range(NCORES)
    ]
    _r = run_bass_kernel_spmd(nc, in_maps, list(range(NCORES)))
    globals()["LAST_RESULTS"] = _r
    res = _r.results

    z = np.concatenate([r["z"] for r in res], axis=0).reshape(B, C, HH, WW)
    idx = np.concatenate([r["idx"].reshape(-1) for r in res]).astype(np.int32)
    ssq = float(sum(r["stats"].astype(np.float64).sum() for r in res))

    n_tokens = B * TOK
    loss = np.float32(BETA * ssq / (n_tokens * C))
    counts = np.bincount(idx, minlength=K).astype(np.float64)
    avg = counts / n_tokens
    perplexity = np.float32(np.exp(-np.sum(avg * np.log(avg + 1e-10))))
    return z, loss, perplexity, idx[:, None].astype(np.int32)


# revision 9
# speedup vs baseline: 2.8421x; 1.1843x over previous
"""VQ codebook (CodebookEMA forward) Trainium2 kernel.

Full inputs -> shard batch axis over 8 NeuronCores (2 images/core) ->
Bass/Tile kernel per core -> gather/assemble full outputs.

Per-core pipeline (per batch image, [256, 4096] d-major token matrix):
  1. PE: scores x.w accumulated in PSUM [128 tok, 1024 codes]
     (codebook transposed on-chip once via PE transposes). Token tiles
     run [128 tokens x 1024 codes] per PSUM tile.
  2. DVE custom scan op: single-pass argmax_k of (x.w - |w|^2/2) read
     straight out of PSUM (bias row broadcast by partition_all_reduce).
  3. GPSIMD indirect DMA: gather codebook rows by token index from DRAM
     ([tok, d] tiles), then PE-transpose to zqT [c, tok] NCHW layout.
  4. DVE custom reduce: commitment loss partials sum((zq - x)^2).
Host: tiny assembly (idx de-permute, loss scalar, perplexity bincount).
"""
import sys

sys.path.insert(0, "/opt/trn_rl_repo")

import numpy as np
from operator import add

import concourse.bass as bass
import concourse.mybir as mybir
import concourse.tile as tile
import concourse.bass_isa as bass_isa
from concourse import bacc
from concourse.bass_utils import run_bass_kernel_spmd
from concourse.masks import make_identity

# problem constants (hardcoded per contract)
B, C, HH, WW = 16, 256, 64, 64
K = 1024
NCORES = 8
BPC = B // NCORES          # batches per core
TOK = HH * WW              # tokens per batch image
BETA = 0.25
F32 = mybir.dt.float32
F32R = mybir.dt.float32r
U16 = mybir.dt.uint16
I32 = mybir.dt.int32
I16 = mybir.dt.int16

NTT = TOK // 128           # token tiles per batch (32)
NR = 8                     # partition sub-groups (r) per tile
NQ = 16                    # wrapped group width (q)
GCH = 8                    # token tiles per indirect-gather chunk

# ---------------------------------------------------------------- custom ops
_OPS = {}


def _register_ops():
    if _OPS:
        return _OPS
    import concourse.dve_ops as dve_ops
    from concourse.dve_ops import DveOp
    from concourse.dve_spec import (
        Spec, Src0, Src1, C0, Idx, AluOp, MaxNeg, eq, select, maxx, sq, Scan,
        lower, _has_src1,
    )
    from concourse.dve_uop import DveOpSpec

    def make_op(name, spec, subdim=False):
        existing = {o.name: o for o in dve_ops.OPS}
        if name in existing:
            return existing[name]
        opcode = dve_ops._CUSTOM_DVE_ROW_BASE + len(dve_ops.OPS)
        shas = {}
        for ver in ("v3", "v4"):
            shas[ver] = DveOpSpec(
                name=name, opcode=opcode, uops=lower(spec, ver=ver),
                rd1_en=_has_src1(spec),
            ).sha(ver)
        op = DveOp(name, spec, subdim=subdim, uops_sha=shas)
        dve_ops.OPS.append(op)
        dve_ops.CUSTOM_DVE_SPECS[name] = spec
        dve_ops._SUB_OPCODE_FOR_NAME[name] = opcode
        return op

    def _ref_argmax_scan(in0, in1, s0, s1, imm2):
        b = (in0.astype(np.float32) - in1.astype(np.float32)).astype(np.float32)
        r = np.maximum.accumulate(b, axis=-1)
        n = b.shape[-1]
        idxs = np.arange(n, dtype=np.float32)
        marked = np.where(b == r, idxs, -np.finfo(np.float32).max)
        return marked, marked.reshape(marked.shape[0], -1).max(axis=-1, keepdims=True)

    def _ref_subsq_reduce(in0, in1, s0, s1, imm2):
        b = (in0.astype(np.float32) - in1.astype(np.float32)) ** 2
        return b.astype(np.float32), (
            s0 + b.reshape(b.shape[0], -1).sum(axis=-1, keepdims=True)
        )

    _b = Src0 - Src1
    _r = Scan(AluOp.MAX, _b)
    _OPS["argmax"] = make_op(
        "ARGMAX_SCAN_VQ",
        Spec(body=select(eq(_b, _r), Idx, MaxNeg), accum=maxx,
             reference=_ref_argmax_scan),
    )
    _OPS["subsq"] = make_op(
        "SUBSQ_REDUCE_VQ",
        Spec(body=sq(Src0 - Src1), accum=add, accum_init=C0,
             reference=_ref_subsq_reduce),
    )
    return _OPS


# ---------------------------------------------------------------- kernel build
_NC_CACHE = {}


def _build():
    if "nc" in _NC_CACHE:
        return _NC_CACHE["nc"]
    ops = _register_ops()
    nc = bacc.Bacc("TRN2", target_bir_lowering=False, debug=False,
                   num_devices=NCORES)

    x_in = nc.declare_dram_parameter("x", [BPC, C, TOK], F32, isOutput=False)
    cb_in = nc.declare_dram_parameter("cb", [K, C], F32, isOutput=False)
    wt_in = nc.declare_dram_parameter("wtT", [C, K], F32, isOutput=False)
    wb_in = nc.declare_dram_parameter("wbias", [1, K], F32, isOutput=False)
    z_out = nc.declare_dram_parameter("z", [BPC, C, TOK], F32, isOutput=True)
    idx_out = nc.declare_dram_parameter("idx", [BPC, TOK], I32, isOutput=True)
    stats_out = nc.declare_dram_parameter("stats", [128, 1], F32, isOutput=True)

    with tile.TileContext(nc) as tc:
        with tc.tile_pool(name="sbuf", bufs=1) as pool, \
             tc.tile_pool(name="psum", bufs=1, space="PSUM") as psum:
            # ---------------- setup: identity, codebook transpose, bias row
            ident = pool.tile([128, 128], F32, name="ident")
            make_identity(nc, ident[:])

            wt = [pool.tile([128, K], F32, name=f"wt{dt}") for dt in range(2)]
            for dt in range(2):
                nc.sync.dma_start(out=wt[dt][:],
                                  in_=wt_in[dt * 128:(dt + 1) * 128, :])
            bias = pool.tile([128, K], F32, name="bias")
            nc.sync.dma_start(out=bias[0:1, :], in_=wb_in[:])
            nc.gpsimd.partition_broadcast(bias[:], bias[0:1, :], channels=128)

            stats_ssq = pool.tile([128, 2 * BPC], F32, name="stats_ssq")

            xb_all, gbuf_all = [], {}

            def phase2(b):
                zq = [pool.tile([128, TOK], F32, tag=f"zq{ct}",
                                name=f"zq{ct}_{b}", bufs=1) for ct in range(2)]
                for tt in range(NTT):
                    gb = gbuf_all[(b, tt // GCH)]
                    g3 = gb[:].rearrange("p (t d) -> p t d", d=C)
                    for ct in range(2):
                        pz = psum.tile([128, 128], F32, tag="pst",
                                       name=f"pz_{b}_{tt}_{ct}", bufs=2)
                        nc.tensor.transpose(
                            pz[:], g3[:, tt % GCH, ct * 128:(ct + 1) * 128],
                            ident[:])
                        nc.scalar.activation(
                            zq[ct][:, tt * 128:(tt + 1) * 128], pz[:],
                            mybir.ActivationFunctionType.Copy)
                for ct in range(2):
                    nc.sync.dma_start(
                        out=z_out[b, ct * 128:(ct + 1) * 128, :], in_=zq[ct][:])
                    nc.vector._custom_dve(
                        ops["subsq"], out=zq[ct][:], in0=zq[ct][:],
                        in1=xb_all[b][ct][:], s0=0.0,
                        accum_out=stats_ssq[:, 2 * b + ct:2 * b + ct + 1])

            # ---------------- phase 1 per batch: score + argmax + idx plumbing
            for b in range(BPC):
                xb = [pool.tile([128, TOK], F32, tag=f"xb{dt}",
                                name=f"xb{dt}_{b}", bufs=2) for dt in range(2)]
                xb_all.append(xb)
                for dt in range(2):
                    nc.sync.dma_start(out=xb[dt][:],
                                      in_=x_in[b, dt * 128:(dt + 1) * 128, :])

                stats_idx = pool.tile([128, NTT], F32, tag="sidx",
                                      name=f"sidx_{b}", bufs=2)
                stats_i32 = pool.tile([128, NTT], I32, tag="sidx32",
                                      name=f"sidx32_{b}", bufs=2)
                for tt in range(NTT):
                    ps = psum.tile([128, K], F32, tag="ps", name=f"ps_{b}_{tt}",
                                   bufs=2)
                    for kc in range(2):
                        for dt in range(2):
                            nc.tensor.matmul(
                                ps[:, kc * 512:(kc + 1) * 512],
                                xb[dt][:, tt * 128:(tt + 1) * 128],
                                wt[dt][:, kc * 512:(kc + 1) * 512],
                                start=(dt == 0), stop=(dt == 1),
                            )
                    scr = pool.tile([128, K], F32, tag="scr",
                                    name=f"scr_{b}_{tt}", bufs=3)
                    nc.vector._custom_dve(
                        ops["argmax"], out=scr[:], in0=ps[:], in1=bias[:],
                        accum_out=stats_idx[:, tt:tt + 1])
                    if tt % GCH == GCH - 1:
                        cc = tt // GCH
                        sl = slice(cc * GCH, (cc + 1) * GCH)
                        nc.vector.tensor_copy(stats_i32[:, sl],
                                              stats_idx[:, sl])
                        gb = pool.tile([128, GCH * C], F32, tag="gbuf",
                                       name=f"gb_{b}_{cc}", bufs=4)
                        gbuf_all[(b, cc)] = gb
                        g3 = gb[:].rearrange("p (t d) -> p t d", d=C)
                        for j in range(GCH):
                            nc.gpsimd.indirect_dma_start(
                                out=g3[:, j],
                                out_offset=None,
                                in_=cb_in[:],
                                in_offset=bass.IndirectOffsetOnAxis(
                                    ap=stats_i32[:, cc * GCH + j:
                                                 cc * GCH + j + 1], axis=0),
                            )
                phase2(b)

                # idx -> token-major: psi[t, p] = idx(token 128t + p)
                psi = psum.tile([NTT, 128], F32, tag="psi", name=f"psi_{b}",
                                bufs=1)
                nc.tensor.transpose(psi[:], stats_idx[:], ident[:])
                idxT32 = pool.tile([NTT, 128], I32, tag="idxT32",
                                   name=f"idxT32_{b}", bufs=2)
                nc.vector.tensor_copy(idxT32[:], psi[:])
                nc.sync.dma_start(
                    out=idx_out[b].rearrange("(t p) -> t p", p=128),
                    in_=idxT32[:])


            stats_red = pool.tile([128, 1], F32, name="stats_red")
            nc.vector.reduce_sum(stats_red[:], stats_ssq[:],
                                 axis=mybir.AxisListType.X)
            nc.sync.dma_start(out=stats_out[:], in_=stats_red[:])

    nc.compile()
    _NC_CACHE["nc"] = nc
    return nc


# ---------------------------------------------------------------- entry point
def kernel(inputs: np.ndarray, codebook: np.ndarray):
    inputs = np.ascontiguousarray(np.asarray(inputs, dtype=np.float32))
    codebook = np.ascontiguousarray(np.asarray(codebook, dtype=np.float32))
    nc = _build()

    x_flat = inputs.reshape(B, C, TOK)
    wtT = np.ascontiguousarray(codebook.T)
    wbias = np.ascontiguousarray(
        (0.5 * (codebook.astype(np.float64) ** 2).sum(axis=1))
        .astype(np.float32)[None, :])
    in_maps = [
        {"x": np.ascontiguousarray(x_flat[c * BPC:(c + 1) * BPC]),
         "cb": codebook, "wtT": wtT, "wbias": wbias}
        for c in range(NCORES)
    ]
    _r = run_bass_kernel_spmd(nc, in_maps, list(range(NCORES)))
    globals()["LAST_RESULTS"] = _r
    res = _r.results

    z = np.concatenate([r["z"] for r in res], axis=0).reshape(B, C, HH, WW)
    idx = np.concatenate([r["idx"].reshape(-1) for r in res]).astype(np.int32)
    ssq = float(sum(r["stats"].astype(np.float64).sum() for r in res))

    n_tokens = B * TOK
    loss = np.float32(BETA * ssq / (n_tokens * C))
    counts = np.bincount(idx, minlength=K).astype(np.float64)
    avg = counts / n_tokens
    perplexity = np.float32(np.exp(-np.sum(avg * np.log(avg + 1e-10))))
    return z, loss, perplexity, idx[:, None].astype(np.int32)


# revision 12
# speedup vs baseline: 3.1998x; 1.1259x over previous
"""VQ codebook (CodebookEMA forward) Trainium2 kernel.

Full inputs -> shard batch axis over 8 NeuronCores (2 images/core) ->
Bass/Tile kernel per core -> gather/assemble full outputs.

Per-core pipeline (per batch image, [256, 4096] d-major token matrix):
  1. PE: scores x.w accumulated in PSUM [128 tok, 1024 codes]
     (codebook transposed on-chip once via PE transposes). Token tiles
     run [128 tokens x 1024 codes] per PSUM tile.
  2. DVE custom scan op: single-pass argmax_k of (x.w - |w|^2/2) read
     straight out of PSUM (bias row broadcast by partition_all_reduce).
  3. GPSIMD indirect DMA: gather codebook rows by token index from DRAM
     ([tok, d] tiles), then PE-transpose to zqT [c, tok] NCHW layout.
  4. DVE custom reduce: commitment loss partials sum((zq - x)^2).
Host: tiny assembly (idx de-permute, loss scalar, perplexity bincount).
"""
import sys

sys.path.insert(0, "/opt/trn_rl_repo")

import numpy as np
from operator import add

import concourse.bass as bass
import concourse.mybir as mybir
import concourse.tile as tile
import concourse.bass_isa as bass_isa
from concourse import bacc
from concourse.bass_utils import run_bass_kernel_spmd
from concourse.masks import make_identity

# problem constants (hardcoded per contract)
B, C, HH, WW = 16, 256, 64, 64
K = 1024
NCORES = 8
BPC = B // NCORES          # batches per core
TOK = HH * WW              # tokens per batch image
BETA = 0.25
F32 = mybir.dt.float32
F32R = mybir.dt.float32r
F16 = mybir.dt.float16
U16 = mybir.dt.uint16
I32 = mybir.dt.int32
I16 = mybir.dt.int16

NTT = TOK // 128           # token tiles per batch (32)
NR = 8                     # partition sub-groups (r) per tile
NQ = 16                    # wrapped group width (q)
GCH = 8                    # token tiles per indirect-gather chunk

# ---------------------------------------------------------------- custom ops
_OPS = {}


def _register_ops():
    if _OPS:
        return _OPS
    import concourse.dve_ops as dve_ops
    from concourse.dve_ops import DveOp
    from concourse.dve_spec import (
        Spec, Src0, Src1, C0, Idx, AluOp, MaxNeg, eq, select, maxx, sq, Scan,
        lower, _has_src1,
    )
    from concourse.dve_uop import DveOpSpec

    def make_op(name, spec, subdim=False):
        existing = {o.name: o for o in dve_ops.OPS}
        if name in existing:
            return existing[name]
        opcode = dve_ops._CUSTOM_DVE_ROW_BASE + len(dve_ops.OPS)
        shas = {}
        for ver in ("v3", "v4"):
            shas[ver] = DveOpSpec(
                name=name, opcode=opcode, uops=lower(spec, ver=ver),
                rd1_en=_has_src1(spec),
            ).sha(ver)
        op = DveOp(name, spec, subdim=subdim, uops_sha=shas)
        dve_ops.OPS.append(op)
        dve_ops.CUSTOM_DVE_SPECS[name] = spec
        dve_ops._SUB_OPCODE_FOR_NAME[name] = opcode
        return op

    def _ref_argmax_scan(in0, in1, s0, s1, imm2):
        b = (in0.astype(np.float32) - in1.astype(np.float32)).astype(np.float32)
        r = np.maximum.accumulate(b, axis=-1)
        n = b.shape[-1]
        idxs = np.arange(n, dtype=np.float32)
        marked = np.where(b == r, idxs, -np.finfo(np.float32).max)
        return marked, marked.reshape(marked.shape[0], -1).max(axis=-1, keepdims=True)

    def _ref_subsq_reduce(in0, in1, s0, s1, imm2):
        b = (in0.astype(np.float32) - in1.astype(np.float32)) ** 2
        return b.astype(np.float32), (
            s0 + b.reshape(b.shape[0], -1).sum(axis=-1, keepdims=True)
        )

    _b = Src0 - Src1
    _r = Scan(AluOp.MAX, _b)
    _OPS["argmax"] = make_op(
        "ARGMAX_SCAN_VQ",
        Spec(body=select(eq(_b, _r), Idx, MaxNeg), accum=maxx,
             reference=_ref_argmax_scan),
    )
    _OPS["subsq"] = make_op(
        "SUBSQ_REDUCE_VQ",
        Spec(body=sq(Src0 - Src1), accum=add, accum_init=C0,
             reference=_ref_subsq_reduce),
    )
    return _OPS


# ---------------------------------------------------------------- kernel build
_NC_CACHE = {}


def _build():
    if "nc" in _NC_CACHE:
        return _NC_CACHE["nc"]
    ops = _register_ops()
    nc = bacc.Bacc("TRN2", target_bir_lowering=False, debug=False,
                   num_devices=NCORES)

    x_in = nc.declare_dram_parameter("x", [BPC, C, TOK], F32, isOutput=False)
    cb_in = nc.declare_dram_parameter("cb", [K, C], F32, isOutput=False)
    wt_in = nc.declare_dram_parameter("wtT", [C, K], F32, isOutput=False)
    wb_in = nc.declare_dram_parameter("wbias", [1, K], F32, isOutput=False)
    z_out = nc.declare_dram_parameter("z", [BPC, C, TOK], F32, isOutput=True)
    idx_out = nc.declare_dram_parameter("idx", [BPC, TOK], I32, isOutput=True)
    stats_out = nc.declare_dram_parameter("stats", [128, 1], F32, isOutput=True)

    with tile.TileContext(nc) as tc:
        with tc.tile_pool(name="sbuf", bufs=1) as pool, \
             tc.tile_pool(name="psum", bufs=1, space="PSUM") as psum:
            # ---------------- setup: identity, codebook transpose, bias row
            ident = pool.tile([128, 128], F32, name="ident")
            make_identity(nc, ident[:])

            wh = [pool.tile([128, K], F16, name=f"wh{dt}") for dt in range(2)]
            wl = [pool.tile([128, K], F16, name=f"wl{dt}") for dt in range(2)]
            for dt in range(2):
                wtmp = pool.tile([128, K], F32, tag="gbuf",
                                 name=f"wtmp{dt}", bufs=4)
                nc.sync.dma_start(out=wtmp[:],
                                  in_=wt_in[dt * 128:(dt + 1) * 128, :])
                nc.scalar.activation(wh[dt][:], wtmp[:],
                                     mybir.ActivationFunctionType.Copy)
                nc.vector.tensor_tensor(wl[dt][:], wtmp[:], wh[dt][:],
                                        op=mybir.AluOpType.subtract)
            bias = pool.tile([128, K], F32, name="bias")
            nc.sync.dma_start(out=bias[0:1, :], in_=wb_in[:])
            nc.gpsimd.partition_broadcast(bias[:], bias[0:1, :], channels=128)

            stats_ssq = pool.tile([128, 2 * BPC], F32, name="stats_ssq")

            xb_all, gbuf_all = [], {}

            def phase2(b):
                zq = [pool.tile([128, TOK], F32, tag=f"zq{ct}",
                                name=f"zq{ct}_{b}", bufs=1) for ct in range(2)]
                for tt in range(NTT):
                    gb = gbuf_all[(b, tt // GCH)]
                    g3 = gb[:].rearrange("p (t d) -> p t d", d=C)
                    for ct in range(2):
                        pz = psum.tile([128, 128], F32, tag="pst",
                                       name=f"pz_{b}_{tt}_{ct}", bufs=2)
                        nc.tensor.transpose(
                            pz[:], g3[:, tt % GCH, ct * 128:(ct + 1) * 128],
                            ident[:])
                        nc.scalar.activation(
                            zq[ct][:, tt * 128:(tt + 1) * 128], pz[:],
                            mybir.ActivationFunctionType.Copy)
                for ct in range(2):
                    nc.sync.dma_start(
                        out=z_out[b, ct * 128:(ct + 1) * 128, :], in_=zq[ct][:])
                    nc.vector._custom_dve(
                        ops["subsq"], out=zq[ct][:], in0=zq[ct][:],
                        in1=xb_all[b][ct][:], s0=0.0,
                        accum_out=stats_ssq[:, 2 * b + ct:2 * b + ct + 1])

            # ---------------- phase 1 per batch: score + argmax + idx plumbing
            for b in range(BPC):
                xb = [pool.tile([128, TOK], F32, tag=f"xb{dt}",
                                name=f"xb{dt}_{b}", bufs=2) for dt in range(2)]
                xb_all.append(xb)
                xh = [pool.tile([128, TOK], F16, tag=f"xh{dt}",
                                name=f"xh{dt}_{b}", bufs=2) for dt in range(2)]
                xl = [pool.tile([128, TOK], F16, tag=f"xl{dt}",
                                name=f"xl{dt}_{b}", bufs=2) for dt in range(2)]
                for dt in range(2):
                    nc.sync.dma_start(out=xb[dt][:],
                                      in_=x_in[b, dt * 128:(dt + 1) * 128, :])
                    nc.scalar.activation(xh[dt][:], xb[dt][:],
                                         mybir.ActivationFunctionType.Copy)
                    nc.vector.tensor_tensor(xl[dt][:], xb[dt][:], xh[dt][:],
                                            op=mybir.AluOpType.subtract)

                stats_idx = pool.tile([128, NTT], F32, tag="sidx",
                                      name=f"sidx_{b}", bufs=2)
                stats_i32 = pool.tile([128, NTT], I32, tag="sidx32",
                                      name=f"sidx32_{b}", bufs=2)
                for tt in range(NTT):
                    ps = psum.tile([128, K], F32, tag="ps", name=f"ps_{b}_{tt}",
                                   bufs=2)
                    ts = slice(tt * 128, (tt + 1) * 128)
                    # fp16 hi/lo split: x.w = xh.wh + xh.wl + xl.wh,
                    # grouped by stationary operand (4 LDWEIGHTS / tile)
                    plan = []
                    for dt in range(2):
                        for kc in range(2):
                            plan.append((xh[dt], wh[dt], kc))
                            plan.append((xh[dt], wl[dt], kc))
                    for dt in range(2):
                        for kc in range(2):
                            plan.append((xl[dt], wh[dt], kc))
                    cnt = {0: 0, 1: 0}
                    for _, _, kc in plan:
                        cnt[kc] += 1
                    seen = {0: 0, 1: 0}
                    for lhs, rhs, kc in plan:
                        seen[kc] += 1
                        nc.tensor.matmul(
                            ps[:, kc * 512:(kc + 1) * 512],
                            lhs[:, ts],
                            rhs[:, kc * 512:(kc + 1) * 512],
                            start=(seen[kc] == 1), stop=(seen[kc] == cnt[kc]),
                        )
                    nc.vector._custom_dve(
                        ops["argmax"], out=ps[:], in0=ps[:], in1=bias[:],
                        accum_out=stats_idx[:, tt:tt + 1])
                    if tt % GCH == GCH - 1:
                        cc = tt // GCH
                        sl = slice(cc * GCH, (cc + 1) * GCH)
                        nc.vector.tensor_copy(stats_i32[:, sl],
                                              stats_idx[:, sl])
                        gb = pool.tile([128, GCH * C], F32, tag="gbuf",
                                       name=f"gb_{b}_{cc}", bufs=4)
                        gbuf_all[(b, cc)] = gb
                        g3 = gb[:].rearrange("p (t d) -> p t d", d=C)
                        for j in range(GCH):
                            nc.gpsimd.indirect_dma_start(
                                out=g3[:, j],
                                out_offset=None,
                                in_=cb_in[:],
                                in_offset=bass.IndirectOffsetOnAxis(
                                    ap=stats_i32[:, cc * GCH + j:
                                                 cc * GCH + j + 1], axis=0),
                            )
                phase2(b)

                # idx -> token-major: psi[t, p] = idx(token 128t + p)
                psi = psum.tile([NTT, 128], F32, tag="psi", name=f"psi_{b}",
                                bufs=1)
                nc.tensor.transpose(psi[:], stats_idx[:], ident[:])
                idxT32 = pool.tile([NTT, 128], I32, tag="idxT32",
                                   name=f"idxT32_{b}", bufs=2)
                nc.vector.tensor_copy(idxT32[:], psi[:])
                nc.sync.dma_start(
                    out=idx_out[b].rearrange("(t p) -> t p", p=128),
                    in_=idxT32[:])


            stats_red = pool.tile([128, 1], F32, name="stats_red")
            nc.vector.reduce_sum(stats_red[:], stats_ssq[:],
                                 axis=mybir.AxisListType.X)
            nc.sync.dma_start(out=stats_out[:], in_=stats_red[:])

    nc.compile()
    _NC_CACHE["nc"] = nc
    return nc


# ---------------------------------------------------------------- entry point
def kernel(inputs: np.ndarray, codebook: np.ndarray):
    inputs = np.ascontiguousarray(np.asarray(inputs, dtype=np.float32))
    codebook = np.ascontiguousarray(np.asarray(codebook, dtype=np.float32))
    nc = _build()

    x_flat = inputs.reshape(B, C, TOK)
    wtT = np.ascontiguousarray(codebook.T)
    wbias = np.ascontiguousarray(
        (0.5 * (codebook.astype(np.float64) ** 2).sum(axis=1))
        .astype(np.float32)[None, :])
    in_maps = [
        {"x": np.ascontiguousarray(x_flat[c * BPC:(c + 1) * BPC]),
         "cb": codebook, "wtT": wtT, "wbias": wbias}
        for c in range(NCORES)
    ]
    _r = run_bass_kernel_spmd(nc, in_maps, list(range(NCORES)))
    globals()["LAST_RESULTS"] = _r
    res = _r.results

    z = np.concatenate([r["z"] for r in res], axis=0).reshape(B, C, HH, WW)
    idx = np.concatenate([r["idx"].reshape(-1) for r in res]).astype(np.int32)
    ssq = float(sum(r["stats"].astype(np.float64).sum() for r in res))

    n_tokens = B * TOK
    loss = np.float32(BETA * ssq / (n_tokens * C))
    counts = np.bincount(idx, minlength=K).astype(np.float64)
    avg = counts / n_tokens
    perplexity = np.float32(np.exp(-np.sum(avg * np.log(avg + 1e-10))))
    return z, loss, perplexity, idx[:, None].astype(np.int32)


# revision 13
# speedup vs baseline: 3.5123x; 1.0976x over previous
"""VQ codebook (CodebookEMA forward) Trainium2 kernel.

Full inputs -> shard batch axis over 8 NeuronCores (2 images/core) ->
Bass/Tile kernel per core -> gather/assemble full outputs.

Per-core pipeline (per batch image, [256, 4096] d-major token matrix):
  1. PE: scores x.w accumulated in PSUM [128 tok, 1024 codes]
     (codebook transposed on-chip once via PE transposes). Token tiles
     run [128 tokens x 1024 codes] per PSUM tile.
  2. DVE custom scan op: single-pass argmax_k of (x.w - |w|^2/2) read
     straight out of PSUM (bias row broadcast by partition_all_reduce).
  3. GPSIMD indirect DMA: gather codebook rows by token index from DRAM
     ([tok, d] tiles), then PE-transpose to zqT [c, tok] NCHW layout.
  4. DVE custom reduce: commitment loss partials sum((zq - x)^2).
Host: tiny assembly (idx de-permute, loss scalar, perplexity bincount).
"""
import sys

sys.path.insert(0, "/opt/trn_rl_repo")

import numpy as np
from operator import add

import concourse.bass as bass
import concourse.mybir as mybir
import concourse.tile as tile
import concourse.bass_isa as bass_isa
from concourse import bacc
from concourse.bass_utils import run_bass_kernel_spmd
from concourse.masks import make_identity

# problem constants (hardcoded per contract)
B, C, HH, WW = 16, 256, 64, 64
K = 1024
NCORES = 8
BPC = B // NCORES          # batches per core
TOK = HH * WW              # tokens per batch image
BETA = 0.25
F32 = mybir.dt.float32
F32R = mybir.dt.float32r
F16 = mybir.dt.float16
U16 = mybir.dt.uint16
I32 = mybir.dt.int32
I16 = mybir.dt.int16

NTT = TOK // 128           # token tiles per batch (32)
NR = 8                     # partition sub-groups (r) per tile
NQ = 16                    # wrapped group width (q)
GCH = 8                    # token tiles per indirect-gather chunk

# ---------------------------------------------------------------- custom ops
_OPS = {}


def _register_ops():
    if _OPS:
        return _OPS
    import concourse.dve_ops as dve_ops
    from concourse.dve_ops import DveOp
    from concourse.dve_spec import (
        Spec, Src0, Src1, C0, Idx, AluOp, MaxNeg, eq, select, maxx, sq, Scan,
        lower, _has_src1,
    )
    from concourse.dve_uop import DveOpSpec

    def make_op(name, spec, subdim=False):
        existing = {o.name: o for o in dve_ops.OPS}
        if name in existing:
            return existing[name]
        opcode = dve_ops._CUSTOM_DVE_ROW_BASE + len(dve_ops.OPS)
        shas = {}
        for ver in ("v3", "v4"):
            shas[ver] = DveOpSpec(
                name=name, opcode=opcode, uops=lower(spec, ver=ver),
                rd1_en=_has_src1(spec),
            ).sha(ver)
        op = DveOp(name, spec, subdim=subdim, uops_sha=shas)
        dve_ops.OPS.append(op)
        dve_ops.CUSTOM_DVE_SPECS[name] = spec
        dve_ops._SUB_OPCODE_FOR_NAME[name] = opcode
        return op

    def _ref_argmax_scan(in0, in1, s0, s1, imm2):
        b = (in0.astype(np.float32) - in1.astype(np.float32)).astype(np.float32)
        r = np.maximum.accumulate(b, axis=-1)
        n = b.shape[-1]
        idxs = np.arange(n, dtype=np.float32)
        marked = np.where(b == r, idxs, -np.finfo(np.float32).max)
        return marked, marked.reshape(marked.shape[0], -1).max(axis=-1, keepdims=True)

    def _ref_subsq_reduce(in0, in1, s0, s1, imm2):
        b = (in0.astype(np.float32) - in1.astype(np.float32)) ** 2
        return b.astype(np.float32), (
            s0 + b.reshape(b.shape[0], -1).sum(axis=-1, keepdims=True)
        )

    _b = Src0 - Src1
    _r = Scan(AluOp.MAX, _b)
    _OPS["argmax"] = make_op(
        "ARGMAX_SCAN_VQ",
        Spec(body=select(eq(_b, _r), Idx, MaxNeg), accum=maxx,
             reference=_ref_argmax_scan),
    )
    _OPS["subsq"] = make_op(
        "SUBSQ_REDUCE_VQ",
        Spec(body=sq(Src0 - Src1), accum=add, accum_init=C0,
             reference=_ref_subsq_reduce),
    )
    return _OPS


# ---------------------------------------------------------------- kernel build
_NC_CACHE = {}


def _build():
    if "nc" in _NC_CACHE:
        return _NC_CACHE["nc"]
    ops = _register_ops()
    nc = bacc.Bacc("TRN2", target_bir_lowering=False, debug=False,
                   num_devices=NCORES)

    x_in = nc.declare_dram_parameter("x", [BPC, C, TOK], F32, isOutput=False)
    cb_in = nc.declare_dram_parameter("cb", [K, C], F32, isOutput=False)
    wt_in = nc.declare_dram_parameter("wtT", [C, K], F32, isOutput=False)
    wb_in = nc.declare_dram_parameter("wbias", [1, K], F32, isOutput=False)
    z_out = nc.declare_dram_parameter("z", [BPC, C, TOK], F32, isOutput=True)
    idx_out = nc.declare_dram_parameter("idx", [BPC, TOK], I32, isOutput=True)
    stats_out = nc.declare_dram_parameter("stats", [128, 1], F32, isOutput=True)

    with tile.TileContext(nc) as tc:
        with tc.tile_pool(name="sbuf", bufs=1) as pool, \
             tc.tile_pool(name="psum", bufs=1, space="PSUM") as psum:
            # ---------------- setup: identity, codebook transpose, bias row
            ident = pool.tile([128, 128], F32, name="ident")
            make_identity(nc, ident[:])

            wh = [pool.tile([128, K], F16, name=f"wh{dt}") for dt in range(2)]
            wl = [pool.tile([128, K], F16, name=f"wl{dt}") for dt in range(2)]
            for dt in range(2):
                wtmp = pool.tile([128, K], F32, tag="gbuf",
                                 name=f"wtmp{dt}", bufs=4)
                nc.sync.dma_start(out=wtmp[:],
                                  in_=wt_in[dt * 128:(dt + 1) * 128, :])
                nc.scalar.activation(wh[dt][:], wtmp[:],
                                     mybir.ActivationFunctionType.Copy)
                nc.vector.tensor_tensor(wl[dt][:], wtmp[:], wh[dt][:],
                                        op=mybir.AluOpType.subtract)
            bias = pool.tile([128, K], F32, name="bias")
            nc.sync.dma_start(out=bias[0:1, :], in_=wb_in[:])
            nc.gpsimd.partition_broadcast(bias[:], bias[0:1, :], channels=128)

            stats_ssq = pool.tile([128, 2 * BPC * (NTT // GCH)], F32,
                                  name="stats_ssq")

            xb_all, gbuf_all = [], {}

            def phase2(b):
                zq = [pool.tile([128, TOK], F32, tag=f"zq{ct}",
                                name=f"zq{ct}_{b}", bufs=1) for ct in range(2)]
                nchunk = NTT // GCH
                for cc in range(nchunk):
                    gb = gbuf_all[(b, cc)]
                    g3 = gb[:].rearrange("p (t d) -> p t d", d=C)
                    cs = slice(cc * GCH * 128, (cc + 1) * GCH * 128)
                    for tt in range(cc * GCH, (cc + 1) * GCH):
                        for ct in range(2):
                            pz = psum.tile([128, 128], F32, tag="pst",
                                           name=f"pz_{b}_{tt}_{ct}", bufs=2)
                            nc.tensor.transpose(
                                pz[:], g3[:, tt % GCH,
                                          ct * 128:(ct + 1) * 128],
                                ident[:])
                            nc.scalar.activation(
                                zq[ct][:, tt * 128:(tt + 1) * 128], pz[:],
                                mybir.ActivationFunctionType.Copy)
                    for ct in range(2):
                        col = (2 * b + ct) * nchunk + cc
                        nc.sync.dma_start(
                            out=z_out[b, ct * 128:(ct + 1) * 128, cs],
                            in_=zq[ct][:, cs])
                        nc.vector._custom_dve(
                            ops["subsq"], out=zq[ct][:, cs],
                            in0=zq[ct][:, cs],
                            in1=xb_all[b][ct][:, cs], s0=0.0,
                            accum_out=stats_ssq[:, col:col + 1])

            # ---------------- phase 1 per batch: score + argmax + idx plumbing
            for b in range(BPC):
                xb = [pool.tile([128, TOK], F32, tag=f"xb{dt}",
                                name=f"xb{dt}_{b}", bufs=2) for dt in range(2)]
                xb_all.append(xb)
                xh = [pool.tile([128, TOK], F16, tag=f"xh{dt}",
                                name=f"xh{dt}_{b}", bufs=2) for dt in range(2)]
                xl = [pool.tile([128, TOK], F16, tag=f"xl{dt}",
                                name=f"xl{dt}_{b}", bufs=2) for dt in range(2)]
                NXQ = 4
                for q in range(NXQ):
                    qs = slice(q * (TOK // NXQ), (q + 1) * (TOK // NXQ))
                    for dt in range(2):
                        nc.sync.dma_start(
                            out=xb[dt][:, qs],
                            in_=x_in[b, dt * 128:(dt + 1) * 128, qs])
                        nc.scalar.activation(
                            xh[dt][:, qs], xb[dt][:, qs],
                            mybir.ActivationFunctionType.Copy)
                        nc.vector.tensor_tensor(
                            xl[dt][:, qs], xb[dt][:, qs], xh[dt][:, qs],
                            op=mybir.AluOpType.subtract)

                stats_idx = pool.tile([128, NTT], F32, tag="sidx",
                                      name=f"sidx_{b}", bufs=2)
                stats_i32 = pool.tile([128, NTT], I32, tag="sidx32",
                                      name=f"sidx32_{b}", bufs=2)
                for tt in range(NTT):
                    ps = psum.tile([128, K], F32, tag="ps", name=f"ps_{b}_{tt}",
                                   bufs=2)
                    ts = slice(tt * 128, (tt + 1) * 128)
                    # fp16 hi/lo split: x.w = xh.wh + xh.wl + xl.wh,
                    # grouped by stationary operand (4 LDWEIGHTS / tile)
                    plan = []
                    for dt in range(2):
                        for kc in range(2):
                            plan.append((xh[dt], wh[dt], kc))
                            plan.append((xh[dt], wl[dt], kc))
                    for dt in range(2):
                        for kc in range(2):
                            plan.append((xl[dt], wh[dt], kc))
                    cnt = {0: 0, 1: 0}
                    for _, _, kc in plan:
                        cnt[kc] += 1
                    seen = {0: 0, 1: 0}
                    for lhs, rhs, kc in plan:
                        seen[kc] += 1
                        nc.tensor.matmul(
                            ps[:, kc * 512:(kc + 1) * 512],
                            lhs[:, ts],
                            rhs[:, kc * 512:(kc + 1) * 512],
                            start=(seen[kc] == 1), stop=(seen[kc] == cnt[kc]),
                        )
                    nc.vector._custom_dve(
                        ops["argmax"], out=ps[:], in0=ps[:], in1=bias[:],
                        accum_out=stats_idx[:, tt:tt + 1])
                    if tt % GCH == GCH - 1:
                        cc = tt // GCH
                        sl = slice(cc * GCH, (cc + 1) * GCH)
                        nc.vector.tensor_copy(stats_i32[:, sl],
                                              stats_idx[:, sl])
                        gb = pool.tile([128, GCH * C], F32, tag="gbuf",
                                       name=f"gb_{b}_{cc}", bufs=4)
                        gbuf_all[(b, cc)] = gb
                        g3 = gb[:].rearrange("p (t d) -> p t d", d=C)
                        for j in range(GCH):
                            nc.gpsimd.indirect_dma_start(
                                out=g3[:, j],
                                out_offset=None,
                                in_=cb_in[:],
                                in_offset=bass.IndirectOffsetOnAxis(
                                    ap=stats_i32[:, cc * GCH + j:
                                                 cc * GCH + j + 1], axis=0),
                            )
                phase2(b)

                # idx -> token-major: psi[t, p] = idx(token 128t + p)
                psi = psum.tile([NTT, 128], F32, tag="psi", name=f"psi_{b}",
                                bufs=1)
                nc.tensor.transpose(psi[:], stats_idx[:], ident[:])
                idxT32 = pool.tile([NTT, 128], I32, tag="idxT32",
                                   name=f"idxT32_{b}", bufs=2)
                nc.vector.tensor_copy(idxT32[:], psi[:])
                nc.sync.dma_start(
                    out=idx_out[b].rearrange("(t p) -> t p", p=128),
                    in_=idxT32[:])


            stats_red = pool.tile([128, 1], F32, name="stats_red")
            nc.vector.reduce_sum(stats_red[:], stats_ssq[:],
                                 axis=mybir.AxisListType.X)
            nc.sync.dma_start(out=stats_out[:], in_=stats_red[:])

    nc.compile()
    _NC_CACHE["nc"] = nc
    return nc


# ---------------------------------------------------------------- entry point
def kernel(inputs: np.ndarray, codebook: np.ndarray):
    inputs = np.ascontiguousarray(np.asarray(inputs, dtype=np.float32))
    codebook = np.ascontiguousarray(np.asarray(codebook, dtype=np.float32))
    nc = _build()

    x_flat = inputs.reshape(B, C, TOK)
    wtT = np.ascontiguousarray(codebook.T)
    wbias = np.ascontiguousarray(
        (0.5 * (codebook.astype(np.float64) ** 2).sum(axis=1))
        .astype(np.float32)[None, :])
    in_maps = [
        {"x": np.ascontiguousarray(x_flat[c * BPC:(c + 1) * BPC]),
         "cb": codebook, "wtT": wtT, "wbias": wbias}
        for c in range(NCORES)
    ]
    _r = run_bass_kernel_spmd(nc, in_maps, list(range(NCORES)))
    globals()["LAST_RESULTS"] = _r
    res = _r.results

    z = np.concatenate([r["z"] for r in res], axis=0).reshape(B, C, HH, WW)
    idx = np.concatenate([r["idx"].reshape(-1) for r in res]).astype(np.int32)
    ssq = float(sum(r["stats"].astype(np.float64).sum() for r in res))

    n_tokens = B * TOK
    loss = np.float32(BETA * ssq / (n_tokens * C))
    counts = np.bincount(idx, minlength=K).astype(np.float64)
    avg = counts / n_tokens
    perplexity = np.float32(np.exp(-np.sum(avg * np.log(avg + 1e-10))))
    return z, loss, perplexity, idx[:, None].astype(np.int32)
